# revision 12
# baseline (speedup 1.0000x reference)
"""Trainium2 Bass kernel for nn_CompleteAttention_68418829025814.

Linformer-style windowed attention. Structure (per core, 4 batches):
  - window_reverse folded into a host-side column permutation of E_w/F_w
    and a host-side permutation of the gathered output.
  - k/v never materialized: k_lowT = Wk^T.T @ (E@x)^T + const (token
    contraction uses x in native layout); q path uses host-transposed x.
  - q bias folded into the q matmul via an augmented ones-row in x^T and a
    bias row in Wq^T (contraction over 193).
  - scores (s^T, [R, tok]) exp'd on ScalarE in 2-bank [128, 2W] chunks.
  - attn@v and denominators col-packed 4+2 heads into 2-bank PSUM tiles;
    softmax denominator reciprocal via DVE reciprocal_approx_fast (single
    pass) instead of the 8-cycle iterative reciprocal; one big normalize
    multiply per tile.
  - projection computed transposed (out[co, tok]) with four small fixed
    stationaries; proj bias applied during the PSUM->SBUF copy as a
    per-partition scalar add; host transposes the final output.
  - all PSUM rotates through a single 4-buf pool of 2-bank slots; phase
    A2(b+1) is interleaved with phase B(b) so the in-order per-engine
    queues stay overlapped.

Sharding: data-parallel over batch; each of the 8 cores gets 4 batches
(256 windows) of x. Small weights are replicated.
"""

import numpy as np

B_TOT = 32
N_CORES = 8
B_PER = B_TOT // N_CORES      # 4 batches per core
N = 3136                      # tokens per batch
NP = 3200                     # padded tokens per batch (6*512 + 128)
C = 192
H = 6
HD = 32
R = 128
WS = 7

_STATE = {}


def _window_perm():
    """n_of_m[m] = spatial index n for window-order position m."""
    hh, ww, i, j = np.meshgrid(
        np.arange(8), np.arange(8), np.arange(7), np.arange(7), indexing="ij"
    )
    m = (hh * 8 + ww) * 49 + i * 7 + j
    n = (hh * 7 + i) * 56 + ww * 7 + j
    n_of_m = np.empty(N, dtype=np.int64)
    n_of_m[m.ravel()] = n.ravel()
    return n_of_m


def _build_bass():
    import concourse.bacc as bacc
    import concourse.mybir as mybir
    from concourse.tile import TileContext

    f32 = mybir.dt.float32
    f16 = mybir.dt.float16
    ADD = mybir.AluOpType.add
    MUL = mybir.AluOpType.mult
    EXP = mybir.ActivationFunctionType.Exp
    IDENT = mybir.ActivationFunctionType.Identity

    nc = bacc.Bacc("TRN2", target_bir_lowering=False, debug=False)

    x_d = nc.dram_tensor("x", [B_PER * NP, C], f16, kind="ExternalInput")
    xt_d = nc.dram_tensor("xt_aug", [C + 1, B_PER * NP], f16, kind="ExternalInput")
    e_d = nc.dram_tensor("e_wxt", [N, R], f16, kind="ExternalInput")
    f_d = nc.dram_tensor("f_wxt", [N, R], f16, kind="ExternalInput")
    wqt_d = nc.dram_tensor("wqt_aug", [C + 1, C], f16, kind="ExternalInput")
    wkt_d = nc.dram_tensor("wkt", [C, C], f16, kind="ExternalInput")
    wvt_d = nc.dram_tensor("wvt", [C, C], f16, kind="ExternalInput")
    ckt_d = nc.dram_tensor("const_kt", [C, R], f32, kind="ExternalInput")
    cv_d = nc.dram_tensor("const_v", [R, C], f32, kind="ExternalInput")
    pwt_d = nc.dram_tensor("pwt", [C, C], f16, kind="ExternalInput")
    pb_d = nc.dram_tensor("pb", [C, 1], f32, kind="ExternalInput")
    ident_d = nc.dram_tensor("ident", [128, 128], f16, kind="ExternalInput")
    ones_d = nc.dram_tensor("ones_att", [128, 32], f16, kind="ExternalInput")
    out_d = nc.dram_tensor("out", [C, B_PER * NP], f32, kind="ExternalOutput")

    NCH = 25  # n-chunks per batch for the E/F contraction (24*128 + 64)

    with TileContext(nc) as tc:
        with tc.tile_pool(name="const", bufs=1) as cpool, \
             tc.tile_pool(name="ef", bufs=1) as efpool, \
             tc.tile_pool(name="low", bufs=1) as lowpool, \
             tc.tile_pool(name="qt", bufs=1) as qtpool, \
             tc.tile_pool(name="xin", bufs=3) as xpool, \
             tc.tile_pool(name="xt", bufs=2) as xtpool, \
             tc.tile_pool(name="sp", bufs=2) as sppool, \
             tc.tile_pool(name="div", bufs=2) as divpool, \
             tc.tile_pool(name="av", bufs=2) as avpool, \
             tc.tile_pool(name="osb", bufs=3) as opool, \
             tc.tile_pool(name="ps", bufs=4, space="PSUM") as ps:

            # ---- constants ----
            ident = cpool.tile([128, 128], f16)
            nc.sync.dma_start(ident[:], ident_d[:])
            wq_hi = cpool.tile([128, C], f16)
            nc.sync.dma_start(wq_hi[:], wqt_d[0:128, :])
            wq_lo = cpool.tile([65, C], f16)
            nc.sync.dma_start(wq_lo[:], wqt_d[128:193, :])
            wkt_h = cpool.tile([128, C], f16)
            nc.gpsimd.dma_start(wkt_h[:], wkt_d[0:128, :])
            wkt_l = cpool.tile([64, C], f16)
            nc.gpsimd.dma_start(wkt_l[:], wkt_d[128:192, :])
            wvt_h = cpool.tile([128, C], f16)
            nc.gpsimd.dma_start(wvt_h[:], wvt_d[0:128, :])
            wvt_l = cpool.tile([64, C], f16)
            nc.gpsimd.dma_start(wvt_l[:], wvt_d[128:192, :])
            ckt_h = cpool.tile([128, R], f32)
            nc.gpsimd.dma_start(ckt_h[:], ckt_d[0:128, :])
            ckt_l = cpool.tile([64, R], f32)
            nc.gpsimd.dma_start(ckt_l[:], ckt_d[128:192, :])
            cv_sb = cpool.tile([128, C], f32)
            nc.gpsimd.dma_start(cv_sb[:], cv_d[:])
            pwa = cpool.tile([128, 128], f16)
            nc.gpsimd.dma_start(pwa[:], pwt_d[0:128, 0:128])
            pwb = cpool.tile([64, 128], f16)
            nc.gpsimd.dma_start(pwb[:], pwt_d[128:192, 0:128])
            pwc = cpool.tile([128, 64], f16)
            nc.gpsimd.dma_start(pwc[:], pwt_d[0:128, 128:192])
            pwd = cpool.tile([64, 64], f16)
            nc.gpsimd.dma_start(pwd[:], pwt_d[128:192, 128:192])
            pb_hi = cpool.tile([128, 1], f32)
            nc.gpsimd.dma_start(pb_hi[:], pb_d[0:128, :])
            pb_lo = cpool.tile([64, 1], f32)
            nc.gpsimd.dma_start(pb_lo[:], pb_d[128:192, :])
            ones_att = cpool.tile([128, 32], f16)
            nc.gpsimd.dma_start(ones_att[:], ones_d[:])

            # E/F transposed weights resident in SBUF: 24 full chunks + tail
            e_sb = efpool.tile([128, 24, 128], f16)
            f_sb = efpool.tile([128, 24, 128], f16)
            for sl in range(4):
                nc.sync.dma_start(
                    e_sb[:, 6 * sl : 6 * sl + 6, :],
                    e_d[768 * sl : 768 * sl + 768, :].rearrange("(k p) r -> p k r", p=128),
                )
                nc.sync.dma_start(
                    f_sb[:, 6 * sl : 6 * sl + 6, :],
                    f_d[768 * sl : 768 * sl + 768, :].rearrange("(k p) r -> p k r", p=128),
                )
            e_tl = efpool.tile([64, 128], f16)
            nc.sync.dma_start(e_tl[:], e_d[3072:3136, :])
            f_tl = efpool.tile([64, 128], f16)
            nc.sync.dma_start(f_tl[:], f_d[3072:3136, :])

            # per-batch low-rank tensors (kept resident across phase B)
            klo_h = [lowpool.tile([128, R], f16, name=f"klo_h{b}") for b in range(B_PER)]
            klo_l = [lowpool.tile([64, R], f16, name=f"klo_l{b}") for b in range(B_PER)]
            vlo = [lowpool.tile([128, C], f16, name=f"vlo{b}") for b in range(B_PER)]
            qth = [qtpool.tile([128, NP], f16, name=f"qth{b}") for b in range(B_PER)]
            qtl = [qtpool.tile([64, NP], f16, name=f"qtl{b}") for b in range(B_PER)]

            xall = x_d.rearrange("(b n) c -> n b c", b=B_PER)

            # -------- Phase A2: q projection (bias folded via ones row) ------
            def qproj(b, t):
                W = 512 if t < 6 else 128
                base = b * NP + t * 512
                tok = t * 512
                xt_h = xtpool.tile([128, W], f16, name="xt_h", tag="xt_h")
                nc.sync.dma_start(xt_h[:], xt_d[0:128, base : base + W])
                xt_l = xtpool.tile([65, W], f16, name="xt_l", tag="xt_l")
                nc.sync.dma_start(xt_l[:], xt_d[128:193, base : base + W])
                q_all = ps.tile([128, 2, 512], f32, name="q_all", tag="b2")
                nc.tensor.matmul(q_all[:, 0, 0:W], wq_hi[:, 0:128], xt_h[:], start=True, stop=False)
                nc.tensor.matmul(q_all[:, 0, 0:W], wq_lo[:, 0:128], xt_l[:], start=False, stop=True)
                nc.tensor.matmul(q_all[0:64, 1, 0:W], wq_hi[:, 128:192], xt_h[:], start=True, stop=False)
                nc.tensor.matmul(q_all[0:64, 1, 0:W], wq_lo[:, 128:192], xt_l[:], start=False, stop=True)
                nc.scalar.copy(qth[b][:, tok : tok + W], q_all[:, 0, 0:W])
                nc.vector.tensor_copy(qtl[b][:, tok : tok + W], q_all[0:64, 1, 0:W])

            for t in range(7):
                qproj(0, t)

            # ---------------- Phase A: EP/FP + low-rank k/v projections ------
            # halves padded to 512 so each stays inside one PSUM bank
            ep_all = ps.tile([128, 2, 512], f32, name="ep_all", tag="b2")
            fp_all = ps.tile([128, 2, 512], f32, name="fp_all", tag="b2")
            for ci in range(NCH):
                nk = 128 if ci < 24 else 64
                x4 = xpool.tile([nk, B_PER, C], f16, name="x4", tag="x4")
                nc.sync.dma_start(x4[:], xall[ci * 128 : ci * 128 + nk, :, :])
                elh = e_sb[:, ci, :] if ci < 24 else e_tl[:]
                flh = f_sb[:, ci, :] if ci < 24 else f_tl[:]
                x4f = x4[:].rearrange("p b c -> p (b c)")
                st, sp = (ci == 0), (ci == NCH - 1)
                nc.tensor.matmul(ep_all[:, 0, 0 : 2 * C], elh, x4f[:, 0 : 2 * C], start=st, stop=sp)
                nc.tensor.matmul(ep_all[:, 1, 0 : 2 * C], elh, x4f[:, 2 * C : 4 * C], start=st, stop=sp)
                nc.tensor.matmul(fp_all[:, 0, 0 : 2 * C], flh, x4f[:, 0 : 2 * C], start=st, stop=sp)
                nc.tensor.matmul(fp_all[:, 1, 0 : 2 * C], flh, x4f[:, 2 * C : 4 * C], start=st, stop=sp)
            ep_sb = xpool.tile([128, 2, 2 * C], f16, name="ep_sb", tag="ep_sb", bufs=1)
            nc.vector.tensor_copy(ep_sb[:], ep_all[:, :, 0 : 2 * C])
            fp_sb = xpool.tile([128, 2, 2 * C], f16, name="fp_sb", tag="fp_sb", bufs=1)
            nc.vector.tensor_copy(fp_sb[:], fp_all[:, :, 0 : 2 * C])

            for b in range(B_PER):
                p2, b2 = b // 2, b % 2
                # transpose EP, FP slices for batch b: (r=128, c=192) -> (c, r)
                tp_e = ps.tile([128, 2, 512], f16, name="tp_e", tag="b2", padded_shape=None)
                nc.tensor.transpose(tp_e[:, 0, 0:128], ep_sb[:, p2, b2 * C : b2 * C + 128], ident[:])
                nc.tensor.transpose(tp_e[0:64, 1, 0:128], ep_sb[:, p2, b2 * C + 128 : b2 * C + 192], ident[:])
                ept_h = xpool.tile([128, 128], f16, name="ept_h", tag="ept_h", bufs=2)
                nc.vector.tensor_copy(ept_h[:], tp_e[:, 0, 0:128])
                ept_l = xpool.tile([64, 128], f16, name="ept_l", tag="ept_l", bufs=2)
                nc.vector.tensor_copy(ept_l[:], tp_e[0:64, 1, 0:128])
                tp_f = ps.tile([128, 2, 512], f16, name="tp_f", tag="b2", padded_shape=None)
                nc.tensor.transpose(tp_f[:, 0, 0:128], fp_sb[:, p2, b2 * C : b2 * C + 128], ident[:])
                nc.tensor.transpose(tp_f[0:64, 1, 0:128], fp_sb[:, p2, b2 * C + 128 : b2 * C + 192], ident[:])
                fpt_h = xpool.tile([128, 128], f16, name="fpt_h", tag="fpt_h", bufs=2)
                nc.vector.tensor_copy(fpt_h[:], tp_f[:, 0, 0:128])
                fpt_l = xpool.tile([64, 128], f16, name="fpt_l", tag="fpt_l", bufs=2)
                nc.vector.tensor_copy(fpt_l[:], tp_f[0:64, 1, 0:128])

                # k_lowT = Wk^T.T @ EP^T + const_kT   (feature-major (kch, r))
                kl_all = ps.tile([128, 2, 512], f32, name="kl_all", tag="b2")
                nc.tensor.matmul(kl_all[:, 0, 0:R], wkt_h[:, 0:128], ept_h[:], start=True, stop=False)
                nc.tensor.matmul(kl_all[:, 0, 0:R], wkt_l[:, 0:128], ept_l[:], start=False, stop=True)
                nc.tensor.matmul(kl_all[0:64, 1, 0:R], wkt_h[:, 128:192], ept_h[:], start=True, stop=False)
                nc.tensor.matmul(kl_all[0:64, 1, 0:R], wkt_l[:, 128:192], ept_l[:], start=False, stop=True)
                nc.vector.tensor_tensor(klo_h[b][:], kl_all[:, 0, 0:R], ckt_h[:], op=ADD)
                nc.vector.tensor_tensor(klo_l[b][:], kl_all[0:64, 1, 0:R], ckt_l[:], op=ADD)
                # v_low (R-major (r, vch)) + const
                vl_ps = ps.tile([128, C], f32, name="vl_ps", tag="b2")
                nc.tensor.matmul(vl_ps[:], fpt_h[:], wvt_h[:], start=True, stop=False)
                nc.tensor.matmul(vl_ps[:], fpt_l[:], wvt_l[:], start=False, stop=True)
                nc.vector.tensor_tensor(vlo[b][:], vl_ps[:], cv_sb[:], op=ADD)

            # ---------------- Phase B: attention tiles ----------
            def front(b, t):
                W = 512 if t < 6 else 128
                base = b * NP + t * 512
                tok = t * 512
                # scores (s^T, [R, tok]) + exp, head-pairs in 2-bank tiles
                spts = [None, None, None]
                for pair in (2, 0, 1):
                    s2 = ps.tile([128, 2, 512], f32, name=f"s{pair}", tag="b2")
                    for k in range(2):
                        h = 2 * pair + k
                        if h < 4:
                            nc.tensor.matmul(
                                s2[:, k, 0:W],
                                klo_h[b][32 * h : 32 * h + 32, :],
                                qth[b][32 * h : 32 * h + 32, tok : tok + W],
                                start=True, stop=True,
                                tile_position=(32 * h, 0),
                            )
                        else:
                            hh = h - 4
                            nc.tensor.matmul(
                                s2[:, k, 0:W],
                                klo_l[b][32 * hh : 32 * hh + 32, :],
                                qtl[b][32 * hh : 32 * hh + 32, tok : tok + W],
                                start=True, stop=True,
                                tile_position=(32 * hh, 0),
                            )
                    spt = sppool.tile([128, 2, W], f16, name=f"spt{pair}", tag=f"spt{pair}")
                    nc.scalar.activation(spt[:], s2[:, :, 0:W], EXP)
                    spts[pair] = spt

                return dict(W=W, base=base, b=b, spts=spts)

            def avz(st):
                W, b, spts = st["W"], st["b"], st["spts"]

                def spt_h(h):
                    return spts[h // 2][:, h % 2, :]

                # attn @ v_low (col-packed: heads 0-3 -> bank 0, 4-5 -> bank 1)
                av = ps.tile([128, 2, 512], f32, name="av_all", tag="b2")
                for h in range(4):
                    nc.tensor.matmul(
                        av[32 * h : 32 * h + 32, 0, 0:W],
                        vlo[b][:, 32 * h : 32 * h + 32],
                        spt_h(h),
                        start=True, stop=True,
                        tile_position=(0, 32 * h),
                    )
                for h in range(4, 6):
                    hh = h - 4
                    nc.tensor.matmul(
                        av[32 * hh : 32 * hh + 32, 1, 0:W],
                        vlo[b][:, 32 * h : 32 * h + 32],
                        spt_h(h),
                        start=True, stop=True,
                        tile_position=(0, 32 * hh),
                    )
                # denominators, same packing
                z = ps.tile([128, 2, 512], f32, name="z_all", tag="b2")
                for h in range(4):
                    nc.tensor.matmul(
                        z[32 * h : 32 * h + 32, 0, 0:W],
                        ones_att[:],
                        spt_h(h),
                        start=True, stop=True,
                        tile_position=(0, 32 * h),
                    )
                for h in range(4, 6):
                    hh = h - 4
                    nc.tensor.matmul(
                        z[32 * hh : 32 * hh + 32, 1, 0:W],
                        ones_att[:],
                        spt_h(h),
                        start=True, stop=True,
                        tile_position=(0, 32 * hh),
                    )
                st["av"] = av
                st["z"] = z

            def norm(st):
                W = st["W"]
                rz = divpool.tile([128, 2, W], f32, name="rz", tag="rz")
                nc.vector.reciprocal_approx_fast(rz[:, 0, :], st["z"][:, 0, 0:W])
                nc.vector.reciprocal_approx_fast(rz[0:64, 1, :], st["z"][0:64, 1, 0:W])
                avn = avpool.tile([128, 2, W], f16, name="avn", tag="avn")
                nc.vector.tensor_tensor(avn[:, 0, :], st["av"][:, 0, 0:W], rz[:, 0, :], op=MUL)
                nc.vector.tensor_tensor(avn[0:64, 1, :], st["av"][0:64, 1, 0:W], rz[0:64, 1, :], op=MUL)
                st["avn"] = avn

            def projout(st):
                W, base, avn = st["W"], st["base"], st["avn"]
                o_all = ps.tile([128, 2, 512], f32, name="o_all", tag="b2")
                nc.tensor.matmul(o_all[:, 0, 0:W], pwa[:], avn[:, 0, :], start=True, stop=False)
                nc.tensor.matmul(o_all[:, 0, 0:W], pwb[:], avn[0:64, 1, :], start=False, stop=True)
                nc.tensor.matmul(o_all[0:64, 1, 0:W], pwc[:], avn[:, 0, :], start=True, stop=False)
                nc.tensor.matmul(o_all[0:64, 1, 0:W], pwd[:], avn[0:64, 1, :], start=False, stop=True)
                o1 = opool.tile([128, W], f32, name="o1", tag="o1")
                nc.vector.tensor_scalar(
                    out=o1[:], in0=o_all[:, 0, 0:W],
                    scalar1=pb_hi[:], scalar2=None, op0=ADD,
                )
                o2 = opool.tile([64, W], f32, name="o2", tag="o2")
                nc.vector.tensor_scalar(
                    out=o2[:], in0=o_all[0:64, 1, 0:W],
                    scalar1=pb_lo[:], scalar2=None, op0=ADD,
                )
                nc.sync.dma_start(out_d[0:128, base : base + W], o1[:])
                nc.gpsimd.dma_start(out_d[128:192, base : base + W], o2[:])

            tiles = [(b, t) for b in range(B_PER) for t in range(7)]
            nt = len(tiles)
            sts = {}
            for k in range(nt + 2):
                if k < nt:
                    sts[k] = front(*tiles[k])
                if 0 <= k - 1 < nt:
                    avz(sts[k - 1])
                    norm(sts[k - 1])
                if k - 2 >= 0:
                    projout(sts.pop(k - 2))
                if k + 7 < nt:
                    qproj(*tiles[k + 7])

    nc.compile()
    return nc


def _get_nc():
    if "nc" not in _STATE:
        _STATE["nc"] = _build_bass()
    return _STATE["nc"]


def kernel(x, qkv_w, qkv_b, E_w, E_b, F_w, F_b, proj_w, proj_b, h, w):
    from concourse.bass_utils import run_bass_kernel_spmd

    x = np.asarray(x, dtype=np.float32)
    qkv_w = np.asarray(qkv_w, dtype=np.float32)
    qkv_b = np.asarray(qkv_b, dtype=np.float32)
    E_w = np.asarray(E_w, dtype=np.float32)
    E_b = np.asarray(E_b, dtype=np.float32)
    F_w = np.asarray(F_w, dtype=np.float32)
    F_b = np.asarray(F_b, dtype=np.float32)
    proj_w = np.asarray(proj_w, dtype=np.float32)
    proj_b = np.asarray(proj_b, dtype=np.float32)
    assert int(h) == 56 and int(w) == 56

    n_of_m = _window_perm()
    E_wx = np.ascontiguousarray(E_w[:, n_of_m])
    F_wx = np.ascontiguousarray(F_w[:, n_of_m])

    Wq, Wk, Wv = qkv_w[0:C], qkv_w[C : 2 * C], qkv_w[2 * C : 3 * C]
    bq, bk, bv = qkv_b[0:C], qkv_b[C : 2 * C], qkv_b[2 * C : 3 * C]
    scale = np.float32(1.0 / np.sqrt(HD))

    const_k = np.outer(E_wx.sum(1), bk) + E_b[:, None]      # (128, 192)
    const_v = (np.outer(F_wx.sum(1), bv) + F_b[:, None]).astype(np.float32)

    wqt_aug = np.zeros((C + 1, C), dtype=np.float16)
    wqt_aug[0:C] = (Wq * scale).T
    wqt_aug[C] = bq * scale
    wkt = np.ascontiguousarray(Wk.T).astype(np.float16)
    wvt = np.ascontiguousarray(Wv.T).astype(np.float16)
    ckt = np.ascontiguousarray(const_k.T.astype(np.float32))  # (192, 128)
    pwt = np.ascontiguousarray(proj_w.T).astype(np.float16)   # (ch, co)
    pb = np.ascontiguousarray(proj_b.reshape(C, 1)).astype(np.float32)

    e_wxt = np.ascontiguousarray(E_wx.T).astype(np.float16)  # (3136, 128)
    f_wxt = np.ascontiguousarray(F_wx.T).astype(np.float16)
    ident = np.eye(128, dtype=np.float16)
    ones_att = np.ones((128, 32), dtype=np.float16)

    consts = dict(
        e_wxt=e_wxt, f_wxt=f_wxt, wqt_aug=wqt_aug, wkt=wkt, wvt=wvt,
        const_kt=ckt, const_v=const_v, pwt=pwt, pb=pb,
        ident=ident, ones_att=ones_att,
    )

    # shard x: core i gets batches 4i..4i+4, padded to NP tokens per batch
    xb = x.reshape(B_TOT, 64 * 49, C).astype(np.float16)
    in_maps = []
    for i in range(N_CORES):
        xi = np.zeros((B_PER, NP, C), dtype=np.float16)
        xi[:, 0:N, :] = xb[B_PER * i : B_PER * (i + 1)]
        xi = xi.reshape(B_PER * NP, C)
        xt_aug = np.empty((C + 1, B_PER * NP), dtype=np.float16)
        xt_aug[0:C] = xi.T
        xt_aug[C] = 1.0
        in_maps.append(
            {**consts, "x": xi, "xt_aug": np.ascontiguousarray(xt_aug)}
        )

    nc = _get_nc()
    _STATE["last_in_maps"] = in_maps
    res = run_bass_kernel_spmd(nc, in_maps, core_ids=list(range(N_CORES)))

    out_win = np.empty((B_TOT, N, C), dtype=np.float32)
    for i in range(N_CORES):
        oi = res.results[i]["out"].reshape(C, B_PER, NP)
        out_win[B_PER * i : B_PER * (i + 1)] = (
            oi[:, :, 0:N].transpose(1, 2, 0)
        )
    # window_reverse on the gathered output
    out_sp = (
        out_win.reshape(B_TOT, 8, 8, 7, 7, C)
        .transpose(0, 1, 3, 2, 4, 5)
        .reshape(B_TOT, N, C)
    )
    return np.ascontiguousarray(out_sp)
